# revision 36
# baseline (speedup 1.0000x reference)
"""Trainium2 Bass kernel for: out = X + 1e-4 * softmax((X W^T)(X W^T)^T / sqrt(D)) @ X

N=8192, D=1024, fp32 inputs. 8 NeuronCores, X sharded row-wise (1024 rows/core).

Math: scores = X S X^T / sqrt(D) with S = W^T W (symmetric). Per core i:
  Yt = S @ X_i^T                        (Yt[d, m] = (X_i S)[m, d])
  scores^T block j: st_j[n, m] = sum_d Xt[d, n] Yt[d, m]
  Et = exp(st/32 - 32)   (constant shift; scores <= ~40 so no max pass needed)
  rowsum[m] = sum_n Et[n, m]   via ones-vector matmuls
  PV[m, d] = sum_n Et[n, m] X[n, d]     accumulated over n-blocks
  out = X_i + GAMMA * PV / rowsum

All matmuls run in fp8e5m2 with DoubleRow (K=256 per instruction, 2x bf16
throughput). fp8 is numerically safe here: the logit diagonal dominates every
row by ~30, so softmax is a near-delta whose quantization error cancels in the
normalization; residual error enters only through the GAMMA=1e-4-scaled term.
"""

import numpy as np

N = 8192
D = 1024
NCORES = 8
MC = N // NCORES  # 1024 rows per core
NB = N // 128  # 64 n-blocks
DK = D // 128  # 8 contraction chunks
NP = NB // 2  # 32 n-block pairs
UP = DK // 2  # 4 contraction chunk-pairs
GAMMA = 1e-4
SCALE = 1.0 / 32.0  # 1/sqrt(D)
SHIFT = -32.0  # softmax stability shift (exact softmax invariant)

_COMPILED = None


def _build():
    import concourse.tile as tile
    from concourse import bacc, mybir

    f32 = mybir.dt.float32
    f8 = mybir.dt.float8e5
    DR = mybir.MatmulPerfMode.DoubleRow

    nc = bacc.Bacc("TRN2", target_bir_lowering=False, debug=False, num_devices=NCORES)

    # DRAM inputs (host-prepared layouts, fp8e5m2 except xi)
    # xtq[j, p, u, t, n] = X[j*128 + n, (2*u+t)*128 + p]     (replicated)
    xtq = nc.dram_tensor("xtq", [NB, 128, UP, 2, 128], f8, kind="ExternalInput").ap()
    # xti8[p, v, t, m] = X_i[m, (2*v+t)*128 + p]             (per-core)
    xti8 = nc.dram_tensor("xti8", [128, UP, 2, MC], f8, kind="ExternalInput").ap()
    # w8[p, u, t, b] = W[(2*u+t)*128 + p, b]                 (replicated)
    w8 = nc.dram_tensor("w8", [128, UP, 2, D], f8, kind="ExternalInput").ap()
    # xn8[h, j2, p, t, c] = X[(2*j2+t)*128 + p, h*512 + c]   (replicated)
    xn8 = nc.dram_tensor("xn8", [2, NP, 128, 2, 512], f8, kind="ExternalInput").ap()
    # xi[h, mc, p, c] = X_i[mc*128 + p, h*512 + c]           (per-core, fp32)
    xi = nc.dram_tensor("xi", [2, DK, 128, 512], f32, kind="ExternalInput").ap()
    # scratch + output
    et_dram = nc.dram_tensor("et_scratch", [NP, 128, 2, MC], f8).ap()
    rs_dram = nc.dram_tensor("rs_scratch", [MC], f32).ap()
    # y[h, mc, p, c] = out_i[mc*128 + p, h*512 + c]
    y = nc.dram_tensor("y", [2, DK, 128, 512], f32, kind="ExternalOutput").ap()

    Exp = mybir.ActivationFunctionType.Exp
    Copy = mybir.ActivationFunctionType.Copy

    with tile.TileContext(nc) as tc:
        with (
            tc.tile_pool(name="persist", bufs=1) as persist,
            tc.tile_pool(name="p0_sb", bufs=1) as p0_sb,
            tc.tile_pool(name="p1_xt", bufs=4) as p1_xt,
            tc.tile_pool(name="p1_et", bufs=3) as p1_et,
            tc.tile_pool(name="p1_rssb", bufs=1) as p1_rssb,
            tc.tile_pool(name="p2_et", bufs=8) as p2_et,
            tc.tile_pool(name="p2_xn", bufs=8) as p2_xn,
            tc.tile_pool(name="p2_xi", bufs=8) as p2_xi,
            tc.tile_pool(name="p2_out", bufs=4) as p2_out,
        ):
            # persistent SBUF
            # yt_sb[p, u, t, m] = Yt[(2*u+t)*128 + p, m]
            yt_sb = persist.tile([128, UP, 2, MC], f8)
            ones_sb = persist.tile([128, 2, 16], f8)
            nc.vector.memset(ones_sb, 1.0)
            shift_sb = persist.tile([128, 1], f32)
            nc.vector.memset(shift_sb, SHIFT)
            rg_sb = persist.tile([128, DK], f32)  # GAMMA / rowsum per (p, mc)

            # ---------- Phase 0: S = W^T W, then Yt = S @ X_i^T ----------
            with tc.tile_pool(name="p0_ps", bufs=2, space="PSUM") as p0_ps:
                w_sb = p0_sb.tile([128, UP, 2, D], f8)
                for u in range(UP):
                    nc.sync.dma_start(out=w_sb[:, u], in_=w8[:, u])
                xti_sb = p0_sb.tile([128, UP, 2, MC], f8)
                for v in range(UP):
                    nc.sync.dma_start(out=xti_sb[:, v], in_=xti8[:, v])
                # s_sb[p, v, t, b] = S[(2*v+t)*128 + p, b]
                s_sb = p0_sb.tile([128, UP, 2, D], f8)

                for ac in range(DK):
                    ps = p0_ps.tile([128, D], f32)
                    for u in range(UP):
                        for h in range(2):
                            nc.tensor.matmul(
                                ps[:, h * 512 : (h + 1) * 512],
                                w_sb[:, u, :, ac * 128 : (ac + 1) * 128],
                                w_sb[:, u, :, h * 512 : (h + 1) * 512],
                                start=(u == 0),
                                stop=(u == UP - 1),
                                perf_mode=DR,
                            )
                    nc.scalar.activation(s_sb[:, ac // 2, ac % 2, :], ps, Copy)

                for dc in range(DK):
                    ps = p0_ps.tile([128, MC], f32)
                    for v in range(UP):
                        for h in range(2):
                            nc.tensor.matmul(
                                ps[:, h * 512 : (h + 1) * 512],
                                s_sb[:, v, :, dc * 128 : (dc + 1) * 128],
                                xti_sb[:, v, :, h * 512 : (h + 1) * 512],
                                start=(v == 0),
                                stop=(v == UP - 1),
                                perf_mode=DR,
                            )
                    nc.scalar.activation(yt_sb[:, dc // 2, dc % 2, :], ps, Copy)

            # ---------- Phase 1: scores^T blocks -> exp -> Et + rowsums ----------
            with (
                tc.tile_pool(name="p1_st", bufs=2, space="PSUM") as p1_st,
                tc.tile_pool(name="p1_rs", bufs=1, space="PSUM") as p1_rs,
            ):
                rs_ps = p1_rs.tile([1, MC], f32)
                for j2 in range(NP):
                    et_sb = p1_et.tile([128, 2, MC], f8)
                    xt2_sb = p1_xt.tile([128, 2, UP, 2, 128], f8)
                    nc.sync.dma_start(
                        out=xt2_sb,
                        in_=xtq[2 * j2 : 2 * j2 + 2].rearrange(
                            "t2 p u t n -> p t2 u t n"
                        ),
                    )
                    for t in range(2):
                        j = 2 * j2 + t
                        xt_sb = xt2_sb[:, t]
                        st = p1_st.tile([128, MC], f32)
                        for u in range(UP):
                            for h in range(2):
                                nc.tensor.matmul(
                                    st[:, h * 512 : (h + 1) * 512],
                                    xt_sb[:, u, :, :],
                                    yt_sb[:, u, :, h * 512 : (h + 1) * 512],
                                    start=(u == 0),
                                    stop=(u == UP - 1),
                                    perf_mode=DR,
                                )
                        for h in range(2):
                            nc.scalar.activation(
                                et_sb[:, t, h * 512 : (h + 1) * 512],
                                st[:, h * 512 : (h + 1) * 512],
                                Exp,
                                bias=shift_sb,
                                scale=SCALE,
                            )
                    for h in range(2):
                        nc.tensor.matmul(
                            rs_ps[:, h * 512 : (h + 1) * 512],
                            ones_sb[:, :, 0:1],
                            et_sb[:, :, h * 512 : (h + 1) * 512],
                            start=(j2 == 0),
                            stop=(j2 == NP - 1),
                            perf_mode=DR,
                        )
                    nc.sync.dma_start(out=et_dram[j2], in_=et_sb)

                # evacuate rowsums -> DRAM (reload partition-major)
                rs_sb = p1_rssb.tile([1, MC], f32)
                nc.scalar.activation(rs_sb, rs_ps, Copy)
                nc.sync.dma_start(out=rs_dram, in_=rs_sb)

            # rg_sb[p, mc] = GAMMA / rs[mc*128 + p]
            rs2 = persist.tile([128, DK], f32)
            nc.sync.dma_start(out=rs2, in_=rs_dram.rearrange("(mc p) -> p mc", p=128))
            nc.vector.reciprocal(rg_sb, rs2)
            nc.scalar.mul(rg_sb, rg_sb, GAMMA)

            # ---------- Phase 2: PV[m, d] accumulation + combine ----------
            with tc.tile_pool(name="p2_ps", bufs=1, space="PSUM") as p2_ps:
                for h in range(2):
                    pv = [
                        p2_ps.tile(
                            [128, 512], f32, name=f"pv{mc}", tag=f"pv{mc}"
                        )
                        for mc in range(DK)
                    ]
                    for j2 in range(NP):
                        et_sb = p2_et.tile([128, 2, MC], f8)
                        nc.gpsimd.dma_start(out=et_sb, in_=et_dram[j2])
                        xn_sb = p2_xn.tile([128, 2, 512], f8)
                        nc.gpsimd.dma_start(out=xn_sb, in_=xn8[h, j2])
                        for mc in range(DK):
                            nc.tensor.matmul(
                                pv[mc],
                                et_sb[:, :, mc * 128 : (mc + 1) * 128],
                                xn_sb,
                                start=(j2 == 0),
                                stop=(j2 == NP - 1),
                                perf_mode=DR,
                            )
                    for mc in range(DK):
                        xi_sb = p2_xi.tile([128, 512], f32)
                        nc.gpsimd.dma_start(out=xi_sb, in_=xi[h, mc])
                        yo = p2_out.tile([128, 512], f32)
                        nc.vector.scalar_tensor_tensor(
                            out=yo,
                            in0=pv[mc],
                            scalar=rg_sb[:, mc : mc + 1],
                            in1=xi_sb,
                            op0=mybir.AluOpType.mult,
                            op1=mybir.AluOpType.add,
                        )
                        nc.sync.dma_start(out=y[h, mc], in_=yo)

    nc.compile()
    return nc


def _prep_inputs(X, W_qk):
    import ml_dtypes

    f8 = ml_dtypes.float8_e5m2
    X = np.asarray(X, dtype=np.float32)
    W = np.asarray(W_qk, dtype=np.float32)
    X8 = X.astype(f8)
    # xtq[j, p, u, t, n] = X[j*128 + n, (2*u+t)*128 + p]
    xtq = np.ascontiguousarray(
        X8.reshape(NB, 128, UP, 2, 128).transpose(0, 4, 2, 3, 1)
    )
    # w8[p, u, t, b] = W[(2*u+t)*128 + p, b]
    w8 = np.ascontiguousarray(
        W.astype(f8).reshape(UP, 2, 128, D).transpose(2, 0, 1, 3)
    )
    # xn8[h, j2, p, t, c] = X[(2*j2+t)*128 + p, h*512 + c]
    xn8 = np.ascontiguousarray(
        X8.reshape(NP, 2, 128, 2, 512).transpose(3, 0, 2, 1, 4)
    )

    in_maps = []
    for i in range(NCORES):
        Xi = X[i * MC : (i + 1) * MC]
        # xti8[p, v, t, m] = X_i[m, (2*v+t)*128 + p]
        xti8 = np.ascontiguousarray(
            Xi.astype(f8).reshape(MC, UP, 2, 128).transpose(3, 1, 2, 0)
        )
        # xi[h, mc, p, c] = X_i[mc*128 + p, h*512 + c]
        xi_arr = np.ascontiguousarray(
            Xi.reshape(DK, 128, 2, 512).transpose(2, 0, 1, 3)
        )
        in_maps.append(
            {"xtq": xtq, "xti8": xti8, "w8": w8, "xn8": xn8, "xi": xi_arr}
        )
    return in_maps


def run(X, W_qk, trace=False):
    from concourse.bass_utils import run_bass_kernel_spmd

    global _COMPILED
    if _COMPILED is None:
        _COMPILED = _build()
    res = run_bass_kernel_spmd(
        _COMPILED, _prep_inputs(X, W_qk), core_ids=list(range(NCORES)), trace=trace
    )
    out = np.concatenate(
        [
            res.results[i]["y"].transpose(1, 2, 0, 3).reshape(MC, D)
            for i in range(NCORES)
        ],
        axis=0,
    ).astype(np.float32)
    return out, res


def kernel(X, W_qk):
    out, _ = run(X, W_qk, trace=False)
    return out


# revision 37
# speedup vs baseline: 1.0147x; 1.0147x over previous
"""Trainium2 Bass kernel for: out = X + 1e-4 * softmax((X W^T)(X W^T)^T / sqrt(D)) @ X

N=8192, D=1024, fp32 inputs. 8 NeuronCores, X sharded row-wise (1024 rows/core).

Math: scores = X S X^T / sqrt(D) with S = W^T W (symmetric). Per core i:
  Yt = S @ X_i^T                        (Yt[d, m] = (X_i S)[m, d])
  scores^T block j: st_j[n, m] = sum_d Xt[d, n] Yt[d, m]
  Et = exp(st/32 - 32)   (constant shift; scores <= ~40 so no max pass needed)
  rowsum[m] = sum_n Et[n, m]   via ones-vector matmuls
  PV[m, d] = sum_n Et[n, m] X[n, d]     accumulated over n-blocks
  out = X_i + GAMMA * PV / rowsum

All matmuls run in fp8e5m2 with DoubleRow (K=256 per instruction, 2x bf16
throughput). fp8 is numerically safe here: the logit diagonal dominates every
row by ~30, so softmax is a near-delta whose quantization error cancels in the
normalization; residual error enters only through the GAMMA=1e-4-scaled term.
"""

import numpy as np

N = 8192
D = 1024
NCORES = 8
MC = N // NCORES  # 1024 rows per core
NB = N // 128  # 64 n-blocks
DK = D // 128  # 8 contraction chunks
NP = NB // 2  # 32 n-block pairs
UP = DK // 2  # 4 contraction chunk-pairs
GAMMA = 1e-4
SCALE = 1.0 / 32.0  # 1/sqrt(D)
SHIFT = -32.0  # softmax stability shift (exact softmax invariant)

_COMPILED = None


def _build():
    import concourse.tile as tile
    from concourse import bacc, mybir

    f32 = mybir.dt.float32
    f8 = mybir.dt.float8e5
    DR = mybir.MatmulPerfMode.DoubleRow

    nc = bacc.Bacc("TRN2", target_bir_lowering=False, debug=False, num_devices=NCORES)

    # DRAM inputs (host-prepared layouts, fp8e5m2 except xi)
    # xtq[j, p, u, t, n] = X[j*128 + n, (2*u+t)*128 + p]     (replicated)
    xtq = nc.dram_tensor("xtq", [NB, 128, UP, 2, 128], f8, kind="ExternalInput").ap()
    # xti8[p, v, t, m] = X_i[m, (2*v+t)*128 + p]             (per-core)
    xti8 = nc.dram_tensor("xti8", [128, UP, 2, MC], f8, kind="ExternalInput").ap()
    # w8[p, u, t, b] = W[(2*u+t)*128 + p, b]                 (replicated)
    w8 = nc.dram_tensor("w8", [128, UP, 2, D], f8, kind="ExternalInput").ap()
    # xn8[h, j2, p, t, c] = X[(2*j2+t)*128 + p, h*512 + c]   (replicated)
    xn8 = nc.dram_tensor("xn8", [2, NP, 128, 2, 512], f8, kind="ExternalInput").ap()
    # xi[h, mc, p, c] = X_i[mc*128 + p, h*512 + c]           (per-core, fp32)
    xi = nc.dram_tensor("xi", [2, DK, 128, 512], f32, kind="ExternalInput").ap()
    # scratch + output
    et_dram = nc.dram_tensor("et_scratch", [NP, 128, 2, MC], f8).ap()
    rs_dram = nc.dram_tensor("rs_scratch", [MC], f32).ap()
    # y[h, mc, p, c] = out_i[mc*128 + p, h*512 + c]
    y = nc.dram_tensor("y", [2, DK, 128, 512], f32, kind="ExternalOutput").ap()

    Exp = mybir.ActivationFunctionType.Exp
    Copy = mybir.ActivationFunctionType.Copy

    with tile.TileContext(nc) as tc:
        with (
            tc.tile_pool(name="persist", bufs=1) as persist,
            tc.tile_pool(name="p0_sb", bufs=1) as p0_sb,
            tc.tile_pool(name="p1_xt", bufs=4) as p1_xt,
            tc.tile_pool(name="p1_et", bufs=3) as p1_et,
            tc.tile_pool(name="p1_rssb", bufs=1) as p1_rssb,
            tc.tile_pool(name="p2_et", bufs=8) as p2_et,
            tc.tile_pool(name="p2_xn", bufs=8) as p2_xn,
            tc.tile_pool(name="p2_xi", bufs=8) as p2_xi,
            tc.tile_pool(name="p2_out", bufs=4) as p2_out,
        ):
            # persistent SBUF
            # yt_sb[p, u, t, m] = Yt[(2*u+t)*128 + p, m]
            yt_sb = persist.tile([128, UP, 2, MC], f8)
            ones_sb = persist.tile([128, 2, 16], f8)
            nc.vector.memset(ones_sb, 1.0)
            shift_sb = persist.tile([128, 1], f32)
            nc.vector.memset(shift_sb, SHIFT)
            rg_sb = persist.tile([128, DK], f32)  # GAMMA / rowsum per (p, mc)

            # ---------- Phase 0: S = W^T W, then Yt = S @ X_i^T ----------
            with tc.tile_pool(name="p0_ps", bufs=2, space="PSUM") as p0_ps:
                w_sb = p0_sb.tile([128, UP, 2, D], f8)
                for u in range(UP):
                    nc.sync.dma_start(out=w_sb[:, u], in_=w8[:, u])
                xti_sb = p0_sb.tile([128, UP, 2, MC], f8)
                for v in range(UP):
                    nc.sync.dma_start(out=xti_sb[:, v], in_=xti8[:, v])
                # s_sb[p, v, t, b] = S[(2*v+t)*128 + p, b]
                s_sb = p0_sb.tile([128, UP, 2, D], f8)

                for ac in range(DK):
                    ps = p0_ps.tile([128, D], f32)
                    for u in range(UP):
                        for h in range(2):
                            nc.tensor.matmul(
                                ps[:, h * 512 : (h + 1) * 512],
                                w_sb[:, u, :, ac * 128 : (ac + 1) * 128],
                                w_sb[:, u, :, h * 512 : (h + 1) * 512],
                                start=(u == 0),
                                stop=(u == UP - 1),
                                perf_mode=DR,
                            )
                    nc.scalar.activation(s_sb[:, ac // 2, ac % 2, :], ps, Copy)

                for dc in range(DK):
                    ps = p0_ps.tile([128, MC], f32)
                    for v in range(UP):
                        for h in range(2):
                            nc.tensor.matmul(
                                ps[:, h * 512 : (h + 1) * 512],
                                s_sb[:, v, :, dc * 128 : (dc + 1) * 128],
                                xti_sb[:, v, :, h * 512 : (h + 1) * 512],
                                start=(v == 0),
                                stop=(v == UP - 1),
                                perf_mode=DR,
                            )
                    nc.scalar.activation(yt_sb[:, dc // 2, dc % 2, :], ps, Copy)

            # ---------- Phase 1: scores^T blocks -> exp -> Et + rowsums ----------
            with (
                tc.tile_pool(name="p1_st", bufs=2, space="PSUM") as p1_st,
                tc.tile_pool(name="p1_rs", bufs=1, space="PSUM") as p1_rs,
            ):
                rs_ps = p1_rs.tile([1, MC], f32)
                for j2 in range(NP):
                    et_sb = p1_et.tile([128, 2, MC], f8)
                    xt2_sb = p1_xt.tile([128, 2, UP, 2, 128], f8)
                    nc.sync.dma_start(
                        out=xt2_sb,
                        in_=xtq[2 * j2 : 2 * j2 + 2].rearrange(
                            "t2 p u t n -> p t2 u t n"
                        ),
                    )
                    for t in range(2):
                        j = 2 * j2 + t
                        xt_sb = xt2_sb[:, t]
                        st = p1_st.tile([128, MC], f32)
                        for u in range(UP):
                            for h in range(2):
                                nc.tensor.matmul(
                                    st[:, h * 512 : (h + 1) * 512],
                                    xt_sb[:, u, :, :],
                                    yt_sb[:, u, :, h * 512 : (h + 1) * 512],
                                    start=(u == 0),
                                    stop=(u == UP - 1),
                                    perf_mode=DR,
                                )
                        for h in range(2):
                            nc.scalar.activation(
                                et_sb[:, t, h * 512 : (h + 1) * 512],
                                st[:, h * 512 : (h + 1) * 512],
                                Exp,
                                bias=shift_sb,
                                scale=SCALE,
                            )
                    for h in range(2):
                        nc.tensor.matmul(
                            rs_ps[:, h * 512 : (h + 1) * 512],
                            ones_sb[:, :, 0:1],
                            et_sb[:, :, h * 512 : (h + 1) * 512],
                            start=(j2 == 0),
                            stop=(j2 == NP - 1),
                            perf_mode=DR,
                        )
                    nc.sync.dma_start(out=et_dram[j2], in_=et_sb)

                # evacuate rowsums -> DRAM (reload partition-major)
                rs_sb = p1_rssb.tile([1, MC], f32)
                nc.scalar.activation(rs_sb, rs_ps, Copy)
                nc.sync.dma_start(out=rs_dram, in_=rs_sb)

            # rg_sb[p, mc] = GAMMA / rs[mc*128 + p]
            rs2 = persist.tile([128, DK], f32)
            nc.sync.dma_start(out=rs2, in_=rs_dram.rearrange("(mc p) -> p mc", p=128))
            nc.vector.reciprocal(rg_sb, rs2)
            nc.scalar.mul(rg_sb, rg_sb, GAMMA)

            # ---------- Phase 2: PV[m, d] accumulation + combine ----------
            with tc.tile_pool(name="p2_ps", bufs=1, space="PSUM") as p2_ps:
                for h in range(2):
                    pv = [
                        p2_ps.tile(
                            [128, 512], f32, name=f"pv{mc}", tag=f"pv{mc}"
                        )
                        for mc in range(DK)
                    ]
                    for j2 in range(NP):
                        et_sb = p2_et.tile([128, 2, MC], f8)
                        nc.gpsimd.dma_start(out=et_sb, in_=et_dram[j2])
                        xn_sb = p2_xn.tile([128, 2, 512], f8)
                        nc.gpsimd.dma_start(out=xn_sb, in_=xn8[h, j2])
                        for mc in range(DK):
                            nc.tensor.matmul(
                                pv[mc],
                                et_sb[:, :, mc * 128 : (mc + 1) * 128],
                                xn_sb,
                                start=(j2 == 0),
                                stop=(j2 == NP - 1),
                                perf_mode=DR,
                            )
                    for mc in range(DK):
                        xi_sb = p2_xi.tile([128, 512], f32)
                        nc.gpsimd.dma_start(out=xi_sb, in_=xi[h, mc])
                        t1 = p2_out.tile([128, 512], f32)
                        nc.scalar.activation(
                            t1, pv[mc], Copy, scale=rg_sb[:, mc : mc + 1]
                        )
                        yo = p2_out.tile([128, 512], f32)
                        nc.vector.tensor_add(yo, t1, xi_sb)
                        nc.sync.dma_start(out=y[h, mc], in_=yo)

    nc.compile()
    return nc


def _prep_inputs(X, W_qk):
    import ml_dtypes

    f8 = ml_dtypes.float8_e5m2
    X = np.asarray(X, dtype=np.float32)
    W = np.asarray(W_qk, dtype=np.float32)
    X8 = X.astype(f8)
    # xtq[j, p, u, t, n] = X[j*128 + n, (2*u+t)*128 + p]
    xtq = np.ascontiguousarray(
        X8.reshape(NB, 128, UP, 2, 128).transpose(0, 4, 2, 3, 1)
    )
    # w8[p, u, t, b] = W[(2*u+t)*128 + p, b]
    w8 = np.ascontiguousarray(
        W.astype(f8).reshape(UP, 2, 128, D).transpose(2, 0, 1, 3)
    )
    # xn8[h, j2, p, t, c] = X[(2*j2+t)*128 + p, h*512 + c]
    xn8 = np.ascontiguousarray(
        X8.reshape(NP, 2, 128, 2, 512).transpose(3, 0, 2, 1, 4)
    )

    in_maps = []
    for i in range(NCORES):
        Xi = X[i * MC : (i + 1) * MC]
        # xti8[p, v, t, m] = X_i[m, (2*v+t)*128 + p]
        xti8 = np.ascontiguousarray(
            Xi.astype(f8).reshape(MC, UP, 2, 128).transpose(3, 1, 2, 0)
        )
        # xi[h, mc, p, c] = X_i[mc*128 + p, h*512 + c]
        xi_arr = np.ascontiguousarray(
            Xi.reshape(DK, 128, 2, 512).transpose(2, 0, 1, 3)
        )
        in_maps.append(
            {"xtq": xtq, "xti8": xti8, "w8": w8, "xn8": xn8, "xi": xi_arr}
        )
    return in_maps


def run(X, W_qk, trace=False):
    from concourse.bass_utils import run_bass_kernel_spmd

    global _COMPILED
    if _COMPILED is None:
        _COMPILED = _build()
    res = run_bass_kernel_spmd(
        _COMPILED, _prep_inputs(X, W_qk), core_ids=list(range(NCORES)), trace=trace
    )
    out = np.concatenate(
        [
            res.results[i]["y"].transpose(1, 2, 0, 3).reshape(MC, D)
            for i in range(NCORES)
        ],
        axis=0,
    ).astype(np.float32)
    return out, res


def kernel(X, W_qk):
    out, _ = run(X, W_qk, trace=False)
    return out
